# revision 16
# baseline (speedup 1.0000x reference)
"""AttentionPool2d Trainium2 kernel, 8-core batch-data-parallel (v2).

Math (reference returns only query position 0):
  x' = x + pos_sp  (host-folded), posc = pos0 - mean(pos_sp)
  sums = sum_s x'_s ; xf_m = sums/256 + posc
  q0 = (1/8)(W_q xf_m + b_q)            (the only query needed; 1/8 = attn scale^2)
  u_h = W_k_h^T q0_h  (folds W_k into the query; k never materialized)
  lg_sp = u^T x' ; lg_m = rowsum(lg_sp)/256 + u^T posc
  w = softmax([lg_sp | lg_m]) ; w' = w_sp + w_m/256
  y = x' @ w'^T ; a0 = blockdiag(W_v) y + (W_v posc) w_m
  out = w_c a0 + (w_c b_v + b_c)        (bias added on host)

All-batch (b,h)=128 packed layout after the u stage: one softmax, two PE
transposes, wide-moving matmuls. b_k provably drops out (softmax shift).
"""
import sys
sys.path.insert(0, "/opt/trn_rl_repo")
import numpy as np
import ml_dtypes
from contextlib import ExitStack

from concourse import bacc, tile, mybir
import concourse.bass as bass
from concourse import masks
from concourse.bass_utils import run_bass_kernel_spmd

P = 128
B, C, S2, L = 64, 1024, 256, 257
NH = 16
NCORE, BPC, CT = 8, 8, 8          # cores, batches/core, c-tiles (and q-tiles)
F32R = mybir.dt.float32r
F32 = mybir.dt.float32
BF16 = mybir.dt.bfloat16
AF = mybir.ActivationFunctionType
AX = mybir.AxisListType
OP = mybir.AluOpType
SCL = 1.0 / 8.0                    # (1/ch^0.25)^2 folded into q0


def _body(ctx: ExitStack, tc, d):
    nc = tc.nc
    const = ctx.enter_context(tc.tile_pool(name="const", bufs=1))
    xpool = ctx.enter_context(tc.tile_pool(name="xpool", bufs=1))
    wpool = ctx.enter_context(tc.tile_pool(name="wpool", bufs=1))
    work = ctx.enter_context(tc.tile_pool(name="work", bufs=1))
    acc = ctx.enter_context(tc.tile_pool(name="acc", bufs=1))
    psB = ctx.enter_context(tc.tile_pool(name="psB", bufs=4, space="PSUM"))
    psS = ctx.enter_context(tc.tile_pool(name="psS", bufs=2, space="PSUM"))

    # ---- tiles ----
    xn = xpool.tile([P, BPC, CT, L], BF16)          # x' natural + posc col 256
    xtn = xpool.tile([P, BPC, 2, C], BF16)          # x'^T (s-part)
    wqt = wpool.tile([P, CT, C], BF16)              # (1/2048) W_q^T (c-part, q)
    wkn = wpool.tile([P, CT, C], BF16)              # W_k natural (krow-part, c)
    wvt = wpool.tile([P, CT, C], BF16)              # W_v^T (c-part, vch)
    wct = wpool.tile([P, CT, C], BF16)              # w_c^T (vch-part, o)
    qbias = wpool.tile([1, C], BF16)                # (1/8)(W_q posc + b_q)
    vposc = wpool.tile([1, C], BF16)                # W_v posc

    # ---- DMAs in FIFO priority order (xn first: reduce chain gates q0) ----
    for h in range(4):
        nc.sync.dma_start(xn[:, 2 * h:2 * h + 2], d["xn"].ap()[:, 2 * h:2 * h + 2])
    nc.sync.dma_start(wqt[:], d["wqt"].ap())
    nc.sync.dma_start(qbias[:], d["qbias"].ap())
    nc.sync.dma_start(wkn[:], d["wkn"].ap())
    nc.sync.dma_start(vposc[:], d["vposc"].ap())
    for h in range(2):
        nc.sync.dma_start(xtn[:, 4 * h:4 * h + 4], d["xtn"].ap()[:, 4 * h:4 * h + 4])
    nc.sync.dma_start(wvt[:], d["wvt"].ap())
    nc.sync.dma_start(wct[:], d["wct"].ap())

    identf = const.tile([P, P], F32)
    masks.make_identity(nc, identf[:])
    ident = const.tile([P, P], BF16)
    nc.vector.tensor_copy(ident[:], identf[:])
    ones8 = const.tile([1, BPC], BF16)
    nc.vector.memset(ones8[:], 1.0)

    # ---- stage A: sums over s, xf0 (vector/scalar split, per DMA chunk) ----
    sums = acc.tile([P, BPC, CT], F32R)             # (b, j)
    for h in range(4):
        # vector: batches 2h..2h+2, j 0..5 ; scalar: j 5..8 (accum trick)
        nc.vector.reduce_sum(sums[:, 2 * h:2 * h + 2, 0:5],
                             xn[:, 2 * h:2 * h + 2, 0:5, 0:S2], axis=AX.X)
        for b in (2 * h, 2 * h + 1):
            for j in (5, 6, 7):
                scr = work.tile([P, S2], F32R, tag="scr")
                nc.scalar.activation(scr[:], xn[:, b, j, 0:S2], AF.Copy,
                                     accum_out=sums[:, b, j:j + 1])
    xf0 = acc.tile([P, BPC, CT], BF16)
    nc.vector.tensor_copy(xf0[:], sums[:])

    # ---- q0 (+bias via ones outer-product) ----
    q0p = psS.tile([P, CT, BPC], F32, tag="ps")     # (i, b)
    for i in range(CT):
        for j in range(CT):
            nc.tensor.matmul(q0p[:, i, :], wqt[:, j, i * P:(i + 1) * P],
                             xf0[:, :, j], start=(j == 0), stop=False)
        nc.tensor.matmul(q0p[:, i, :], qbias[0:1, i * P:(i + 1) * P],
                         ones8[:], start=False, stop=True)

    # block-diagonal q0 for the per-head W_k^T fold: col = t*16 + b*2 + h'
    q0blk = acc.tile([P, CT, BPC, 2], BF16)
    nc.vector.memset(q0blk[:], 0.0)
    nc.scalar.activation(q0blk[0:64, :, :, 0], q0p[0:64, :, :], AF.Copy)
    nc.scalar.activation(q0blk[64:P, :, :, 1], q0p[64:P, :, :], AF.Copy)

    # ---- u = blockdiag(W_k)^T q0 ; permuted to (b-major, h) columns ----
    usb = acc.tile([P, CT, BPC, CT, 2], BF16)       # [c-part, j, b, t, h']
    for j in range(CT):
        up = psS.tile([P, CT, BPC, 2], F32, tag="ps")   # (t, b, h')
        for t in range(CT):
            nc.tensor.matmul(up[:, t, :, :], wkn[:, t, j * P:(j + 1) * P],
                             q0blk[:, t, :, :], start=True, stop=True)
        nc.vector.tensor_copy(usb[:, j], up[:].transpose([0, 2, 1, 3]))

    # ---- logits: per-b 16-col stationary into 32-spaced psum blocks ----
    # group g = b//4 holds 4 batches at partition bases 32*(b%4); row =
    # 32*(b%4) + h within a group. Col 256 = u^T posc (posc is x col 256).
    # j-outer, (g,k)-inner: the four 32-col-group positions can overlap on
    # the PE array (column tiling), and each j starts as soon as usb_j lands.
    lgps = [psB.tile([P, 512], F32, tag="pb", name=f"lgp{g}")
            for g in range(2)]
    for j in range(CT):
        for g in range(2):
            for k in range(4):
                b = g * 4 + k
                nc.tensor.matmul(lgps[g][32 * k:32 * k + 16, 0:L],
                                 usb[:, j, b], xn[:, b, j, :],
                                 start=(j == 0), stop=(j == CT - 1),
                                 tile_position=(0, 32 * k))
    lgall = [work.tile([P, L + 3], F32, tag=f"lgall{g}", name=f"lgall{g}")
             for g in range(2)]
    nc.vector.tensor_copy(lgall[0][:, 0:L], lgps[0][:, 0:L])
    nc.scalar.activation(lgall[1][:, 0:L], lgps[1][:, 0:L], AF.Copy)

    # mean-token logit col = rowsum/256 + u^T posc  (then batched softmax)
    wps = []
    wmcols = []
    for g in range(2):
        lg = lgall[g]
        rowsum = work.tile([P, 1], F32, tag=f"rs{g}")
        nc.vector.reduce_sum(rowsum[:], lg[:, 0:S2], axis=AX.X)
        nc.vector.tensor_scalar(lg[:, S2:S2 + 1], rowsum[:], 1.0 / S2,
                                lg[:, S2:S2 + 1], op0=OP.mult, op1=OP.add)
        negmax = work.tile([P, 1], F32, tag=f"nm{g}")
        nc.vector.reduce_max(negmax[:], lg[:, 0:L], axis=AX.X, negate=True)
        ex = work.tile([P, L], F32R, tag=f"ex{g}")
        sumexp = work.tile([P, 1], F32R, tag=f"se{g}")
        nc.scalar.activation(ex[:], lg[:, 0:L], AF.Exp, bias=negmax[:],
                             accum_out=sumexp[:])
        recip = work.tile([P, 1], F32, tag=f"rc{g}")
        nc.vector.reciprocal(recip[:], sumexp[:])
        emdiv = work.tile([P, 1], F32, tag=f"ed{g}")
        nc.vector.tensor_scalar_mul(emdiv[:], ex[:, S2:L], 1.0 / S2)
        wp = work.tile([P, S2], BF16, tag=f"wp{g}")  # w' = (e_sp + e_m/256) r
        nc.vector.tensor_scalar(wp[:], ex[:, 0:S2], emdiv[:], recip[:],
                                op0=OP.add, op1=OP.mult)
        wmcol = work.tile([P, 1], BF16, tag=f"wm{g}")  # w_m = e_m r
        nc.vector.tensor_scalar(wmcol[:], ex[:, S2:L], recip[:], None,
                                op0=OP.mult)
        wps.append(wp)
        wmcols.append(wmcol)

    # ---- transposes: w'^T chunks (s-part, 32-spaced (b,h)) + w_m row ----
    wtas = []
    wmrow = work.tile([1, BPC, 16], BF16)
    for g in range(2):
        wta = acc.tile([P, 2, P], BF16, tag=f"wta{g}")
        for t2 in range(2):
            tp2 = psS.tile([P, P], BF16, tag="ps")
            nc.tensor.transpose(tp2[:], wps[g][:, t2 * P:(t2 + 1) * P],
                                ident[:])
            nc.vector.tensor_copy(wta[:, t2], tp2[:])
        wtas.append(wta)
        wmp = psS.tile([1, 4, 32], BF16, tag="ps")
        nc.tensor.transpose(wmp[0:1], wmcols[g][:], ident[:])
        nc.vector.tensor_copy(wmrow[0:1, g * 4:(g + 1) * 4, :],
                              wmp[0:1, :, 0:16])

    # ---- y^T = w'^T-stationary @ x'^T  -> 32-spaced [(b,h), c] ----
    yTs = [acc.tile([P, C], BF16, tag=f"yT{g}", name=f"yT{g}")
           for g in range(2)]
    for b in range(BPC):
        g, k = b // 4, b % 4
        for half in range(2):
            yp = psB.tile([P, 512], F32, tag="pb")
            for t2 in range(2):
                nc.tensor.matmul(yp[:], wtas[g][:, t2],
                                 xtn[:, b, t2, half * 512:(half + 1) * 512],
                                 start=(t2 == 0), stop=(t2 == 1))
            if b % 2 == 0:
                nc.vector.tensor_copy(
                    yTs[g][32 * k:32 * k + 16, half * 512:(half + 1) * 512],
                    yp[32 * k:32 * k + 16, 0:512])
            else:
                nc.scalar.activation(
                    yTs[g][32 * k:32 * k + 16, half * 512:(half + 1) * 512],
                    yp[32 * k:32 * k + 16, 0:512], AF.Copy)

    # ---- transpose y^T -> y [c-part, (b,h)], compacting 32-spacing ----
    yall = acc.tile([P, CT, BPC, 16], BF16)
    for g in range(2):
        for j in range(CT):
            tp = psS.tile([P, 4, 32], BF16, tag="ps")
            nc.tensor.transpose(tp[:], yTs[g][:, j * P:(j + 1) * P], ident[:])
            nc.vector.tensor_copy(yall[:, j, g * 4:(g + 1) * 4, :],
                                  tp[:, :, 0:16])

    # ---- a0 = blockdiag(W_v) y + vposc * w_m ----
    a0ps = psS.tile([P, CT, BPC, 2], F32, tag="ps")
    for r in range(CT):
        for j in range(CT):
            nc.tensor.matmul(a0ps[:, r, :, :], wvt[:, j, r * P:(r + 1) * P],
                             yall[:, j, :, 2 * r:2 * r + 2],
                             start=(j == 0), stop=False)
        nc.tensor.matmul(a0ps[:, r, :, :], vposc[0:1, r * P:(r + 1) * P],
                         wmrow[0:1, :, 2 * r:2 * r + 2], start=False, stop=True)
    a0blk = acc.tile([P, CT, BPC], BF16)            # (vch-part, (r, b))
    nc.scalar.activation(a0blk[0:64, :, :], a0ps[0:64, :, :, 0], AF.Copy)
    nc.scalar.activation(a0blk[64:P, :, :], a0ps[64:P, :, :, 1], AF.Copy)

    # ---- out^T = a0-stationary @ w_c^T  -> [b, o] ----
    osb = acc.tile([BPC, 2, 512], F32)
    for half in range(2):
        op = psB.tile([P, 512], F32, tag="pb")
        for r in range(CT):
            nc.tensor.matmul(op[0:BPC, :], a0blk[:, r, :],
                             wct[:, r, half * 512:(half + 1) * 512],
                             start=(r == 0), stop=(r == CT - 1))
        nc.vector.tensor_copy(osb[:, half, :], op[0:BPC, :])
    nc.sync.dma_start(d["out"].ap(), osb[:])


_CACHE = {}


def _get_nc():
    if "nc" in _CACHE:
        return _CACHE["nc"]
    nc = bacc.Bacc("TRN2", target_bir_lowering=False, debug=False,
                   num_devices=NCORE)
    d = {}
    d["xn"] = nc.dram_tensor("xn", [P, BPC, CT, L], BF16, kind="ExternalInput")
    d["xtn"] = nc.dram_tensor("xtn", [P, BPC, 2, C], BF16, kind="ExternalInput")
    d["wqt"] = nc.dram_tensor("wqt", [P, CT, C], BF16, kind="ExternalInput")
    d["wkn"] = nc.dram_tensor("wkn", [P, CT, C], BF16, kind="ExternalInput")
    d["wvt"] = nc.dram_tensor("wvt", [P, CT, C], BF16, kind="ExternalInput")
    d["wct"] = nc.dram_tensor("wct", [P, CT, C], BF16, kind="ExternalInput")
    d["qbias"] = nc.dram_tensor("qbias", [1, C], BF16, kind="ExternalInput")
    d["vposc"] = nc.dram_tensor("vposc", [1, C], BF16, kind="ExternalInput")
    d["out"] = nc.dram_tensor("out", [BPC, 2, 512], F32, kind="ExternalOutput")
    with tile.TileContext(nc) as tc, ExitStack() as ctx, \
            nc.allow_low_precision(reason="float32r tiles hold f32 bits"):
        _body(ctx, tc, d)
    nc.compile()
    _CACHE["nc"] = nc
    return nc


def _prep_maps(inputs):
    xf = inputs["x"].reshape(B, C, S2).astype(np.float32)
    pos = inputs["pos_emb"].astype(np.float32)
    xp = xf + pos[None, :, 1:]
    posc = pos[:, 0] - pos[:, 1:].mean(axis=1)          # [C]
    wqkv = inputs["w_qkv"].astype(np.float32)
    wq, wk, wv = wqkv[0:C], wqkv[C:2 * C], wqkv[2 * C:3 * C]
    bq = inputs["b_qkv"][0:C].astype(np.float32)
    bv = inputs["b_qkv"][2 * C:3 * C].astype(np.float32)
    wc = inputs["w_c"].astype(np.float32)
    bc = inputs["b_c"].astype(np.float32)

    bf = ml_dtypes.bfloat16
    wqt = np.ascontiguousarray(
        (wq.T * (SCL / S2)).reshape(CT, P, C).transpose(1, 0, 2)).astype(bf)
    wkn = np.ascontiguousarray(
        wk.reshape(CT, P, C).transpose(1, 0, 2)).astype(bf)
    wvt = np.ascontiguousarray(
        wv.T.reshape(CT, P, C).transpose(1, 0, 2)).astype(bf)
    wct = np.ascontiguousarray(
        wc.T.reshape(CT, P, C).transpose(1, 0, 2)).astype(bf)
    qbias = np.ascontiguousarray((SCL * (wq @ posc + bq))[None, :]).astype(bf)
    vposc = np.ascontiguousarray((wv @ posc)[None, :]).astype(bf)
    poscp = posc.reshape(CT, P).T                        # [P, CT]
    shared = dict(wqt=wqt, wkn=wkn, wvt=wvt, wct=wct,
                  qbias=qbias, vposc=vposc)
    maps = []
    for c in range(NCORE):
        xc = xp[c * BPC:(c + 1) * BPC]                   # [8, 1024, 256]
        m = dict(shared)
        xnc = np.empty((P, BPC, CT, L), dtype=np.float32)
        xnc[:, :, :, 0:S2] = xc.reshape(BPC, CT, P, S2).transpose(2, 0, 1, 3)
        xnc[:, :, :, S2] = poscp[:, None, :]
        m["xn"] = np.ascontiguousarray(xnc).astype(bf)
        m["xtn"] = np.ascontiguousarray(
            xc.reshape(BPC, C, 2, P).transpose(3, 0, 2, 1)).astype(bf)
        maps.append(m)
    _CACHE["hostbias"] = wc @ bv + bc                    # [C]
    return maps


def kernel(**inputs) -> np.ndarray:
    nc = _get_nc()
    maps = _prep_maps(inputs)
    res = run_bass_kernel_spmd(nc, maps, list(range(NCORE)))
    hb = _CACHE["hostbias"]
    outs = []
    for c in range(NCORE):
        arr = res.results[c]["out"].reshape(BPC, C).astype(np.float32)
        outs.append(arr + hb[None, :])
    return np.concatenate(outs, axis=0)


if __name__ == "__main__":
    rng = np.random.default_rng(0)
    ins = {
        "x": rng.standard_normal((B, C, 16, 16), dtype=np.float32),
        "pos_emb": rng.standard_normal((C, L), dtype=np.float32) / 32,
        "w_qkv": rng.standard_normal((3 * C, C), dtype=np.float32) / 32,
        "b_qkv": rng.standard_normal((3 * C,), dtype=np.float32) * 0.1,
        "w_c": rng.standard_normal((C, C), dtype=np.float32) / 32,
        "b_c": rng.standard_normal((C,), dtype=np.float32) * 0.1,
    }
    o = kernel(**ins)
    print("out", o.shape, o.dtype, float(np.abs(o).mean()))


# revision 18
# speedup vs baseline: 1.0436x; 1.0436x over previous
"""AttentionPool2d Trainium2 kernel, 8-core batch-data-parallel (v2).

Math (reference returns only query position 0):
  x' = x + pos_sp  (host-folded), posc = pos0 - mean(pos_sp)
  sums = sum_s x'_s ; xf_m = sums/256 + posc
  q0 = (1/8)(W_q xf_m + b_q)            (the only query needed; 1/8 = attn scale^2)
  u_h = W_k_h^T q0_h  (folds W_k into the query; k never materialized)
  lg_sp = u^T x' ; lg_m = rowsum(lg_sp)/256 + u^T posc
  w = softmax([lg_sp | lg_m]) ; w' = w_sp + w_m/256
  y = x' @ w'^T ; a0 = blockdiag(W_v) y + (W_v posc) w_m
  out = w_c a0 + (w_c b_v + b_c)        (bias added on host)

All-batch (b,h)=128 packed layout after the u stage: one softmax, two PE
transposes, wide-moving matmuls. b_k provably drops out (softmax shift).
"""
import sys
sys.path.insert(0, "/opt/trn_rl_repo")
import numpy as np
import ml_dtypes
from contextlib import ExitStack

from concourse import bacc, tile, mybir
import concourse.bass as bass
from concourse import masks
from concourse.bass_utils import run_bass_kernel_spmd

P = 128
B, C, S2, L = 64, 1024, 256, 257
NH = 16
NCORE, BPC, CT = 8, 8, 8          # cores, batches/core, c-tiles (and q-tiles)
F32R = mybir.dt.float32r
F32 = mybir.dt.float32
BF16 = mybir.dt.bfloat16
AF = mybir.ActivationFunctionType
AX = mybir.AxisListType
OP = mybir.AluOpType
SCL = 1.0 / 8.0                    # (1/ch^0.25)^2 folded into q0


def _body(ctx: ExitStack, tc, d):
    nc = tc.nc
    const = ctx.enter_context(tc.tile_pool(name="const", bufs=1))
    xpool = ctx.enter_context(tc.tile_pool(name="xpool", bufs=1))
    wpool = ctx.enter_context(tc.tile_pool(name="wpool", bufs=1))
    work = ctx.enter_context(tc.tile_pool(name="work", bufs=1))
    acc = ctx.enter_context(tc.tile_pool(name="acc", bufs=1))
    psB = ctx.enter_context(tc.tile_pool(name="psB", bufs=4, space="PSUM"))
    psS = ctx.enter_context(tc.tile_pool(name="psS", bufs=2, space="PSUM"))

    # ---- tiles ----
    xn = xpool.tile([P, BPC, CT, L], BF16)          # x' natural + posc col 256
    xtn = xpool.tile([P, BPC, 2, C], BF16)          # x'^T (s-part)
    wqt = wpool.tile([P, CT, C], BF16)              # (1/2048) W_q^T (c-part, q)
    wkn = wpool.tile([P, CT, C], BF16)              # W_k natural (krow-part, c)
    wvt = wpool.tile([P, CT, C], BF16)              # W_v^T (c-part, vch)
    wct = wpool.tile([P, CT, C], BF16)              # w_c^T (vch-part, o)
    qbias = wpool.tile([1, C], BF16)                # (1/8)(W_q posc + b_q)
    vposc = wpool.tile([1, C], BF16)                # W_v posc

    # ---- DMAs in FIFO priority order (xn first: reduce chain gates q0) ----
    for h in range(4):
        nc.sync.dma_start(xn[:, 2 * h:2 * h + 2], d["xn"].ap()[:, 2 * h:2 * h + 2])
    nc.sync.dma_start(wqt[:], d["wqt"].ap())
    nc.sync.dma_start(qbias[:], d["qbias"].ap())
    nc.sync.dma_start(wkn[:], d["wkn"].ap())
    nc.sync.dma_start(vposc[:], d["vposc"].ap())
    for h in range(2):
        nc.sync.dma_start(xtn[:, 4 * h:4 * h + 4], d["xtn"].ap()[:, 4 * h:4 * h + 4])
    nc.sync.dma_start(wvt[:], d["wvt"].ap())
    nc.sync.dma_start(wct[:], d["wct"].ap())

    identf = const.tile([P, P], F32)
    masks.make_identity(nc, identf[:])
    ident = const.tile([P, P], BF16)
    nc.vector.tensor_copy(ident[:], identf[:])
    ones8 = const.tile([1, BPC], BF16)
    nc.vector.memset(ones8[:], 1.0)

    # ---- stage A: sums over s, xf0 (bf16 pair-add halves reduce work) ----
    sums = acc.tile([P, BPC, CT], F32R)             # (b, j)
    for h in range(4):
        xh = work.tile([P, 2, CT, P], BF16, tag="xh")
        nc.vector.tensor_tensor(xh[:], xn[:, 2 * h:2 * h + 2, :, 0:P],
                                xn[:, 2 * h:2 * h + 2, :, P:S2], op=OP.add)
        nc.vector.reduce_sum(sums[:, 2 * h:2 * h + 2, :], xh[:], axis=AX.X)
    xf0 = acc.tile([P, BPC, CT], BF16)
    nc.vector.tensor_copy(xf0[:], sums[:])

    # ---- q0 (+bias via ones outer-product) ----
    q0p = psS.tile([P, CT, BPC], F32, tag="ps")     # (i, b)
    for i in range(CT):
        for j in range(CT):
            nc.tensor.matmul(q0p[:, i, :], wqt[:, j, i * P:(i + 1) * P],
                             xf0[:, :, j], start=(j == 0), stop=False)
        nc.tensor.matmul(q0p[:, i, :], qbias[0:1, i * P:(i + 1) * P],
                         ones8[:], start=False, stop=True)

    # block-diagonal q0 for the per-head W_k^T fold: col = t*16 + b*2 + h'
    q0blk = acc.tile([P, CT, BPC, 2], BF16)
    nc.vector.memset(q0blk[:], 0.0)
    nc.scalar.activation(q0blk[0:64, :, :, 0], q0p[0:64, :, :], AF.Copy)
    nc.scalar.activation(q0blk[64:P, :, :, 1], q0p[64:P, :, :], AF.Copy)

    # ---- u = blockdiag(W_k)^T q0 ; permuted to (b-major, h) columns ----
    usb = acc.tile([P, CT, BPC, CT, 2], BF16)       # [c-part, j, b, t, h']
    for j in range(CT):
        up = psS.tile([P, CT, BPC, 2], F32, tag="ps")   # (t, b, h')
        for t in range(CT):
            nc.tensor.matmul(up[:, t, :, :], wkn[:, t, j * P:(j + 1) * P],
                             q0blk[:, t, :, :], start=True, stop=True)
        nc.vector.tensor_copy(usb[:, j], up[:].transpose([0, 2, 1, 3]))

    # ---- logits: per-b 16-col stationary into 32-spaced psum blocks ----
    # group g = b//4 holds 4 batches at partition bases 32*(b%4); row =
    # 32*(b%4) + h within a group. Col 256 = u^T posc (posc is x col 256).
    # j-outer, (g,k)-inner: the four 32-col-group positions can overlap on
    # the PE array (column tiling), and each j starts as soon as usb_j lands.
    lgps = [psB.tile([P, 512], F32, tag="pb", name=f"lgp{g}")
            for g in range(2)]
    for g in range(2):
        for j in range(CT):
            for k in range(4):
                b = g * 4 + k
                nc.tensor.matmul(lgps[g][32 * k:32 * k + 16, 0:L],
                                 usb[:, j, b], xn[:, b, j, :],
                                 start=(j == 0), stop=(j == CT - 1),
                                 tile_position=(0, 32 * k))
    lgall = [work.tile([P, L + 3], F32, tag=f"lgall{g}", name=f"lgall{g}")
             for g in range(2)]
    nc.vector.tensor_copy(lgall[0][:, 0:L], lgps[0][:, 0:L])
    nc.scalar.activation(lgall[1][:, 0:L], lgps[1][:, 0:L], AF.Copy)

    # mean-token logit col = rowsum/256 + u^T posc  (then batched softmax)
    wps = []
    wmcols = []
    for g in range(2):
        lg = lgall[g]
        rowsum = work.tile([P, 1], F32, tag=f"rs{g}")
        nc.vector.reduce_sum(rowsum[:], lg[:, 0:S2], axis=AX.X)
        nc.vector.tensor_scalar(lg[:, S2:S2 + 1], rowsum[:], 1.0 / S2,
                                lg[:, S2:S2 + 1], op0=OP.mult, op1=OP.add)
        negmax = work.tile([P, 1], F32, tag=f"nm{g}")
        nc.vector.reduce_max(negmax[:], lg[:, 0:L], axis=AX.X, negate=True)
        ex = work.tile([P, L], F32R, tag=f"ex{g}")
        sumexp = work.tile([P, 1], F32R, tag=f"se{g}")
        nc.scalar.activation(ex[:], lg[:, 0:L], AF.Exp, bias=negmax[:],
                             accum_out=sumexp[:])
        recip = work.tile([P, 1], F32, tag=f"rc{g}")
        nc.vector.reciprocal(recip[:], sumexp[:])
        emdiv = work.tile([P, 1], F32, tag=f"ed{g}")
        nc.vector.tensor_scalar_mul(emdiv[:], ex[:, S2:L], 1.0 / S2)
        wp = work.tile([P, S2], BF16, tag=f"wp{g}")  # w' = (e_sp + e_m/256) r
        nc.vector.tensor_scalar(wp[:], ex[:, 0:S2], emdiv[:], recip[:],
                                op0=OP.add, op1=OP.mult)
        wmcol = work.tile([P, 1], BF16, tag=f"wm{g}")  # w_m = e_m r
        nc.vector.tensor_scalar(wmcol[:], ex[:, S2:L], recip[:], None,
                                op0=OP.mult)
        wps.append(wp)
        wmcols.append(wmcol)

    # ---- transposes: w'^T chunks (s-part, 32-spaced (b,h)) + w_m row ----
    wtas = []
    wmrow = work.tile([1, BPC, 16], BF16)
    for g in range(2):
        wta = acc.tile([P, 2, P], BF16, tag=f"wta{g}")
        for t2 in range(2):
            tp2 = psS.tile([P, P], BF16, tag="ps")
            nc.tensor.transpose(tp2[:], wps[g][:, t2 * P:(t2 + 1) * P],
                                ident[:])
            nc.vector.tensor_copy(wta[:, t2], tp2[:])
        wtas.append(wta)
        wmp = psS.tile([1, 4, 32], BF16, tag="ps")
        nc.tensor.transpose(wmp[0:1], wmcols[g][:], ident[:])
        nc.vector.tensor_copy(wmrow[0:1, g * 4:(g + 1) * 4, :],
                              wmp[0:1, :, 0:16])

    # ---- y^T = w'^T-stationary @ x'^T  -> 32-spaced [(b,h), c] ----
    yTs = [acc.tile([P, C], BF16, tag=f"yT{g}", name=f"yT{g}")
           for g in range(2)]
    for b in range(BPC):
        g, k = b // 4, b % 4
        for half in range(2):
            yp = psB.tile([P, 512], F32, tag="pb")
            for t2 in range(2):
                nc.tensor.matmul(yp[:], wtas[g][:, t2],
                                 xtn[:, b, t2, half * 512:(half + 1) * 512],
                                 start=(t2 == 0), stop=(t2 == 1))
            if b % 2 == 0:
                nc.vector.tensor_copy(
                    yTs[g][32 * k:32 * k + 16, half * 512:(half + 1) * 512],
                    yp[32 * k:32 * k + 16, 0:512])
            else:
                nc.scalar.activation(
                    yTs[g][32 * k:32 * k + 16, half * 512:(half + 1) * 512],
                    yp[32 * k:32 * k + 16, 0:512], AF.Copy)

    # ---- transpose y^T -> y [c-part, (b,h)], compacting 32-spacing ----
    yall = acc.tile([P, CT, BPC, 16], BF16)
    for g in range(2):
        for j in range(CT):
            tp = psS.tile([P, 4, 32], BF16, tag="ps")
            nc.tensor.transpose(tp[:], yTs[g][:, j * P:(j + 1) * P], ident[:])
            nc.vector.tensor_copy(yall[:, j, g * 4:(g + 1) * 4, :],
                                  tp[:, :, 0:16])

    # ---- a0 = blockdiag(W_v) y + vposc * w_m ----
    a0ps = psS.tile([P, CT, BPC, 2], F32, tag="ps")
    for r in range(CT):
        for j in range(CT):
            nc.tensor.matmul(a0ps[:, r, :, :], wvt[:, j, r * P:(r + 1) * P],
                             yall[:, j, :, 2 * r:2 * r + 2],
                             start=(j == 0), stop=False)
        nc.tensor.matmul(a0ps[:, r, :, :], vposc[0:1, r * P:(r + 1) * P],
                         wmrow[0:1, :, 2 * r:2 * r + 2], start=False, stop=True)
    a0blk = acc.tile([P, CT, BPC], BF16)            # (vch-part, (r, b))
    nc.scalar.activation(a0blk[0:64, :, :], a0ps[0:64, :, :, 0], AF.Copy)
    nc.scalar.activation(a0blk[64:P, :, :], a0ps[64:P, :, :, 1], AF.Copy)

    # ---- out^T = a0-stationary @ w_c^T  -> [b, o] ----
    osb = acc.tile([BPC, 2, 512], F32)
    for half in range(2):
        op = psB.tile([P, 512], F32, tag="pb")
        for r in range(CT):
            nc.tensor.matmul(op[0:BPC, :], a0blk[:, r, :],
                             wct[:, r, half * 512:(half + 1) * 512],
                             start=(r == 0), stop=(r == CT - 1))
        nc.vector.tensor_copy(osb[:, half, :], op[0:BPC, :])
    nc.sync.dma_start(d["out"].ap(), osb[:])


_CACHE = {}


def _get_nc():
    if "nc" in _CACHE:
        return _CACHE["nc"]
    nc = bacc.Bacc("TRN2", target_bir_lowering=False, debug=False,
                   num_devices=NCORE)
    d = {}
    d["xn"] = nc.dram_tensor("xn", [P, BPC, CT, L], BF16, kind="ExternalInput")
    d["xtn"] = nc.dram_tensor("xtn", [P, BPC, 2, C], BF16, kind="ExternalInput")
    d["wqt"] = nc.dram_tensor("wqt", [P, CT, C], BF16, kind="ExternalInput")
    d["wkn"] = nc.dram_tensor("wkn", [P, CT, C], BF16, kind="ExternalInput")
    d["wvt"] = nc.dram_tensor("wvt", [P, CT, C], BF16, kind="ExternalInput")
    d["wct"] = nc.dram_tensor("wct", [P, CT, C], BF16, kind="ExternalInput")
    d["qbias"] = nc.dram_tensor("qbias", [1, C], BF16, kind="ExternalInput")
    d["vposc"] = nc.dram_tensor("vposc", [1, C], BF16, kind="ExternalInput")
    d["out"] = nc.dram_tensor("out", [BPC, 2, 512], F32, kind="ExternalOutput")
    with tile.TileContext(nc) as tc, ExitStack() as ctx, \
            nc.allow_low_precision(reason="float32r tiles hold f32 bits"):
        _body(ctx, tc, d)
    nc.compile()
    _CACHE["nc"] = nc
    return nc


def _prep_maps(inputs):
    xf = inputs["x"].reshape(B, C, S2).astype(np.float32)
    pos = inputs["pos_emb"].astype(np.float32)
    xp = xf + pos[None, :, 1:]
    posc = pos[:, 0] - pos[:, 1:].mean(axis=1)          # [C]
    wqkv = inputs["w_qkv"].astype(np.float32)
    wq, wk, wv = wqkv[0:C], wqkv[C:2 * C], wqkv[2 * C:3 * C]
    bq = inputs["b_qkv"][0:C].astype(np.float32)
    bv = inputs["b_qkv"][2 * C:3 * C].astype(np.float32)
    wc = inputs["w_c"].astype(np.float32)
    bc = inputs["b_c"].astype(np.float32)

    bf = ml_dtypes.bfloat16
    wqt = np.ascontiguousarray(
        (wq.T * (SCL / S2)).reshape(CT, P, C).transpose(1, 0, 2)).astype(bf)
    wkn = np.ascontiguousarray(
        wk.reshape(CT, P, C).transpose(1, 0, 2)).astype(bf)
    wvt = np.ascontiguousarray(
        wv.T.reshape(CT, P, C).transpose(1, 0, 2)).astype(bf)
    wct = np.ascontiguousarray(
        wc.T.reshape(CT, P, C).transpose(1, 0, 2)).astype(bf)
    qbias = np.ascontiguousarray((SCL * (wq @ posc + bq))[None, :]).astype(bf)
    vposc = np.ascontiguousarray((wv @ posc)[None, :]).astype(bf)
    poscp = posc.reshape(CT, P).T                        # [P, CT]
    shared = dict(wqt=wqt, wkn=wkn, wvt=wvt, wct=wct,
                  qbias=qbias, vposc=vposc)
    maps = []
    for c in range(NCORE):
        xc = xp[c * BPC:(c + 1) * BPC]                   # [8, 1024, 256]
        m = dict(shared)
        xnc = np.empty((P, BPC, CT, L), dtype=np.float32)
        xnc[:, :, :, 0:S2] = xc.reshape(BPC, CT, P, S2).transpose(2, 0, 1, 3)
        xnc[:, :, :, S2] = poscp[:, None, :]
        m["xn"] = np.ascontiguousarray(xnc).astype(bf)
        m["xtn"] = np.ascontiguousarray(
            xc.reshape(BPC, C, 2, P).transpose(3, 0, 2, 1)).astype(bf)
        maps.append(m)
    _CACHE["hostbias"] = wc @ bv + bc                    # [C]
    return maps


def kernel(**inputs) -> np.ndarray:
    nc = _get_nc()
    maps = _prep_maps(inputs)
    res = run_bass_kernel_spmd(nc, maps, list(range(NCORE)))
    hb = _CACHE["hostbias"]
    outs = []
    for c in range(NCORE):
        arr = res.results[c]["out"].reshape(BPC, C).astype(np.float32)
        outs.append(arr + hb[None, :])
    return np.concatenate(outs, axis=0)


if __name__ == "__main__":
    rng = np.random.default_rng(0)
    ins = {
        "x": rng.standard_normal((B, C, 16, 16), dtype=np.float32),
        "pos_emb": rng.standard_normal((C, L), dtype=np.float32) / 32,
        "w_qkv": rng.standard_normal((3 * C, C), dtype=np.float32) / 32,
        "b_qkv": rng.standard_normal((3 * C,), dtype=np.float32) * 0.1,
        "w_c": rng.standard_normal((C, C), dtype=np.float32) / 32,
        "b_c": rng.standard_normal((C,), dtype=np.float32) * 0.1,
    }
    o = kernel(**ins)
    print("out", o.shape, o.dtype, float(np.abs(o).mean()))


# revision 20
# speedup vs baseline: 1.2159x; 1.1651x over previous
"""AttentionPool2d Trainium2 kernel, 8-core batch-data-parallel (v2).

Math (reference returns only query position 0):
  x' = x + pos_sp  (host-folded), posc = pos0 - mean(pos_sp)
  sums = sum_s x'_s ; xf_m = sums/256 + posc
  q0 = (1/8)(W_q xf_m + b_q)            (the only query needed; 1/8 = attn scale^2)
  u_h = W_k_h^T q0_h  (folds W_k into the query; k never materialized)
  lg_sp = u^T x' ; lg_m = rowsum(lg_sp)/256 + u^T posc
  w = softmax([lg_sp | lg_m]) ; w' = w_sp + w_m/256
  y = x' @ w'^T ; a0 = blockdiag(W_v) y + (W_v posc) w_m
  out = w_c a0 + (w_c b_v + b_c)        (bias added on host)

All-batch (b,h)=128 packed layout after the u stage: one softmax, two PE
transposes, wide-moving matmuls. b_k provably drops out (softmax shift).
"""
import sys
sys.path.insert(0, "/opt/trn_rl_repo")
import numpy as np
import ml_dtypes
from contextlib import ExitStack

from concourse import bacc, tile, mybir
import concourse.bass as bass
from concourse import masks
from concourse.bass_utils import run_bass_kernel_spmd

P = 128
B, C, S2, L = 64, 1024, 256, 257
NH = 16
NCORE, BPC, CT = 8, 8, 8          # cores, batches/core, c-tiles (and q-tiles)
F32R = mybir.dt.float32r
F32 = mybir.dt.float32
BF16 = mybir.dt.bfloat16
AF = mybir.ActivationFunctionType
AX = mybir.AxisListType
OP = mybir.AluOpType
SCL = 1.0 / 8.0                    # (1/ch^0.25)^2 folded into q0


def _body(ctx: ExitStack, tc, d):
    nc = tc.nc
    const = ctx.enter_context(tc.tile_pool(name="const", bufs=1))
    xpool = ctx.enter_context(tc.tile_pool(name="xpool", bufs=1))
    wpool = ctx.enter_context(tc.tile_pool(name="wpool", bufs=1))
    work = ctx.enter_context(tc.tile_pool(name="work", bufs=1))
    acc = ctx.enter_context(tc.tile_pool(name="acc", bufs=1))
    psB = ctx.enter_context(tc.tile_pool(name="psB", bufs=4, space="PSUM"))
    psS = ctx.enter_context(tc.tile_pool(name="psS", bufs=2, space="PSUM"))

    # ---- tiles ----
    xn = xpool.tile([P, BPC, CT, L], BF16)          # x' natural + posc col 256
    xtn = xpool.tile([P, BPC, 2, C], BF16)          # x'^T (s-part)
    wqt = wpool.tile([P, CT, C], BF16)              # (1/2048) W_q^T (c-part, q)
    wkn = wpool.tile([P, CT, C], BF16)              # W_k natural (krow-part, c)
    wvt = wpool.tile([P, CT, C], BF16)              # W_v^T (c-part, vch)
    wct = wpool.tile([P, CT, C], BF16)              # w_c^T (vch-part, o)
    qbias = wpool.tile([1, C], BF16)                # (1/8)(W_q posc + b_q)
    vposc = wpool.tile([1, C], BF16)                # W_v posc

    # ---- DMAs in FIFO priority order (xn first: reduce chain gates q0).
    # Bulk loads issue from the otherwise-idle gpsimd queue; smalls on sync.
    for b in range(BPC):
        nc.gpsimd.dma_start(xn[:, b:b + 1], d["xn"].ap()[:, b:b + 1])
    nc.gpsimd.dma_start(wqt[:], d["wqt"].ap())
    nc.sync.dma_start(qbias[:], d["qbias"].ap())
    nc.gpsimd.dma_start(wkn[:], d["wkn"].ap())
    nc.sync.dma_start(vposc[:], d["vposc"].ap())
    for h in range(2):
        nc.gpsimd.dma_start(xtn[:, 4 * h:4 * h + 4], d["xtn"].ap()[:, 4 * h:4 * h + 4])
    nc.gpsimd.dma_start(wvt[:], d["wvt"].ap())
    nc.gpsimd.dma_start(wct[:], d["wct"].ap())

    identf = const.tile([P, P], F32)
    masks.make_identity(nc, identf[:])
    ident = const.tile([P, P], BF16)
    nc.vector.tensor_copy(ident[:], identf[:])
    ones8 = const.tile([1, BPC], BF16)
    nc.vector.memset(ones8[:], 1.0)

    # ---- stage A: sums over s, xf0 (bf16 pair-add, chunked per batch) ----
    sums = acc.tile([P, BPC, CT], F32R)             # (b, j)
    for b in range(BPC):
        xh = work.tile([P, CT, P], BF16, tag="xh")
        nc.vector.tensor_tensor(xh[:], xn[:, b, :, 0:P],
                                xn[:, b, :, P:S2], op=OP.add)
        nc.vector.reduce_sum(sums[:, b, :], xh[:], axis=AX.X)
    xf0 = acc.tile([P, BPC, CT], BF16)
    nc.vector.tensor_copy(xf0[:], sums[:])

    # ---- q0 (+bias via ones outer-product) ----
    q0p = psS.tile([P, CT, BPC], F32, tag="ps")     # (i, b)
    for i in range(CT):
        for j in range(CT):
            nc.tensor.matmul(q0p[:, i, :], wqt[:, j, i * P:(i + 1) * P],
                             xf0[:, :, j], start=(j == 0), stop=False)
        nc.tensor.matmul(q0p[:, i, :], qbias[0:1, i * P:(i + 1) * P],
                         ones8[:], start=False, stop=True)

    # block-diagonal q0 for the per-head W_k^T fold: col = t*16 + b*2 + h'
    q0blk = acc.tile([P, CT, BPC, 2], BF16)
    nc.vector.memset(q0blk[:], 0.0)
    nc.scalar.activation(q0blk[0:64, :, :, 0], q0p[0:64, :, :], AF.Copy)
    nc.scalar.activation(q0blk[64:P, :, :, 1], q0p[64:P, :, :], AF.Copy)

    # ---- u = blockdiag(W_k)^T q0 ; permuted to (b-major, h) columns ----
    usb = acc.tile([P, CT, BPC, CT, 2], BF16)       # [c-part, j, b, t, h']
    for j in range(CT):
        up = psS.tile([P, CT, BPC, 2], F32, tag="ps")   # (t, b, h')
        for t in range(CT):
            nc.tensor.matmul(up[:, t, :, :], wkn[:, t, j * P:(j + 1) * P],
                             q0blk[:, t, :, :], start=True, stop=True)
        nc.vector.tensor_copy(usb[:, j], up[:].transpose([0, 2, 1, 3]))

    # ---- logits: per-b 16-col stationary into 32-spaced psum blocks ----
    # group g = b//4 holds 4 batches at partition bases 32*(b%4); row =
    # 32*(b%4) + h within a group. Col 256 = u^T posc (posc is x col 256).
    # j-outer, (g,k)-inner: the four 32-col-group positions can overlap on
    # the PE array (column tiling), and each j starts as soon as usb_j lands.
    lgps = [psB.tile([P, 512], F32, tag="pb", name=f"lgp{g}")
            for g in range(2)]
    for g in range(2):
        for j in range(CT):
            for k in range(4):
                b = g * 4 + k
                nc.tensor.matmul(lgps[g][32 * k:32 * k + 16, 0:L],
                                 usb[:, j, b], xn[:, b, j, :],
                                 start=(j == 0), stop=(j == CT - 1),
                                 tile_position=(0, 32 * k))
    lgall = [work.tile([P, L + 3], F32, tag=f"lgall{g}", name=f"lgall{g}")
             for g in range(2)]
    nc.vector.tensor_copy(lgall[0][:, 0:L], lgps[0][:, 0:L])
    nc.scalar.activation(lgall[1][:, 0:L], lgps[1][:, 0:L], AF.Copy)

    # mean-token logit col = rowsum/256 + u^T posc  (then batched softmax)
    wps = []
    wmcols = []
    for g in range(2):
        lg = lgall[g]
        rowsum = work.tile([P, 1], F32, tag=f"rs{g}")
        nc.vector.reduce_sum(rowsum[:], lg[:, 0:S2], axis=AX.X)
        nc.vector.tensor_scalar(lg[:, S2:S2 + 1], rowsum[:], 1.0 / S2,
                                lg[:, S2:S2 + 1], op0=OP.mult, op1=OP.add)
        negmax = work.tile([P, 1], F32, tag=f"nm{g}")
        nc.vector.reduce_max(negmax[:], lg[:, 0:L], axis=AX.X, negate=True)
        ex = work.tile([P, L], F32R, tag=f"ex{g}")
        sumexp = work.tile([P, 1], F32R, tag=f"se{g}")
        nc.scalar.activation(ex[:], lg[:, 0:L], AF.Exp, bias=negmax[:],
                             accum_out=sumexp[:])
        recip = work.tile([P, 1], F32, tag=f"rc{g}")
        nc.vector.reciprocal(recip[:], sumexp[:])
        emdiv = work.tile([P, 1], F32, tag=f"ed{g}")
        nc.vector.tensor_scalar_mul(emdiv[:], ex[:, S2:L], 1.0 / S2)
        wp = work.tile([P, S2], BF16, tag=f"wp{g}")  # w' = (e_sp + e_m/256) r
        nc.vector.tensor_scalar(wp[:], ex[:, 0:S2], emdiv[:], recip[:],
                                op0=OP.add, op1=OP.mult)
        wmcol = work.tile([P, 1], BF16, tag=f"wm{g}")  # w_m = e_m r
        nc.vector.tensor_scalar(wmcol[:], ex[:, S2:L], recip[:], None,
                                op0=OP.mult)
        wps.append(wp)
        wmcols.append(wmcol)

    # ---- transposes: w'^T chunks (s-part, 32-spaced (b,h)) + w_m row ----
    wtas = []
    wmrow = work.tile([1, BPC, 16], BF16)
    for g in range(2):
        wta = acc.tile([P, 2, P], BF16, tag=f"wta{g}")
        for t2 in range(2):
            tp2 = psS.tile([P, P], BF16, tag="ps")
            nc.tensor.transpose(tp2[:], wps[g][:, t2 * P:(t2 + 1) * P],
                                ident[:])
            nc.vector.tensor_copy(wta[:, t2], tp2[:])
        wtas.append(wta)
        wmp = psS.tile([1, 4, 32], BF16, tag="ps")
        nc.tensor.transpose(wmp[0:1], wmcols[g][:], ident[:])
        nc.vector.tensor_copy(wmrow[0:1, g * 4:(g + 1) * 4, :],
                              wmp[0:1, :, 0:16])

    # ---- y^T = w'^T-stationary @ x'^T  -> 32-spaced [(b,h), c] ----
    yTs = [acc.tile([P, C], BF16, tag=f"yT{g}", name=f"yT{g}")
           for g in range(2)]
    for b in range(BPC):
        g, k = b // 4, b % 4
        for half in range(2):
            yp = psB.tile([P, 512], F32, tag="pb")
            for t2 in range(2):
                nc.tensor.matmul(yp[:], wtas[g][:, t2],
                                 xtn[:, b, t2, half * 512:(half + 1) * 512],
                                 start=(t2 == 0), stop=(t2 == 1))
            if b % 2 == 0:
                nc.vector.tensor_copy(
                    yTs[g][32 * k:32 * k + 16, half * 512:(half + 1) * 512],
                    yp[32 * k:32 * k + 16, 0:512])
            else:
                nc.scalar.activation(
                    yTs[g][32 * k:32 * k + 16, half * 512:(half + 1) * 512],
                    yp[32 * k:32 * k + 16, 0:512], AF.Copy)

    # ---- transpose y^T -> y [c-part, (b,h)], compacting 32-spacing ----
    yall = acc.tile([P, CT, BPC, 16], BF16)
    for g in range(2):
        for j in range(CT):
            tp = psS.tile([P, 4, 32], BF16, tag="ps")
            nc.tensor.transpose(tp[:], yTs[g][:, j * P:(j + 1) * P], ident[:])
            nc.vector.tensor_copy(yall[:, j, g * 4:(g + 1) * 4, :],
                                  tp[:, :, 0:16])

    # ---- a0 = blockdiag(W_v) y + vposc * w_m ----
    a0ps = psS.tile([P, CT, BPC, 2], F32, tag="ps")
    for r in range(CT):
        for j in range(CT):
            nc.tensor.matmul(a0ps[:, r, :, :], wvt[:, j, r * P:(r + 1) * P],
                             yall[:, j, :, 2 * r:2 * r + 2],
                             start=(j == 0), stop=False)
        nc.tensor.matmul(a0ps[:, r, :, :], vposc[0:1, r * P:(r + 1) * P],
                         wmrow[0:1, :, 2 * r:2 * r + 2], start=False, stop=True)
    a0blk = acc.tile([P, CT, BPC], BF16)            # (vch-part, (r, b))
    nc.scalar.activation(a0blk[0:64, :, :], a0ps[0:64, :, :, 0], AF.Copy)
    nc.scalar.activation(a0blk[64:P, :, :], a0ps[64:P, :, :, 1], AF.Copy)

    # ---- out^T = a0-stationary @ w_c^T  -> [b, o] ----
    osb = acc.tile([BPC, 2, 512], F32)
    for half in range(2):
        op = psB.tile([P, 512], F32, tag="pb")
        for r in range(CT):
            nc.tensor.matmul(op[0:BPC, :], a0blk[:, r, :],
                             wct[:, r, half * 512:(half + 1) * 512],
                             start=(r == 0), stop=(r == CT - 1))
        nc.vector.tensor_copy(osb[:, half, :], op[0:BPC, :])
    nc.sync.dma_start(d["out"].ap(), osb[:])


_CACHE = {}


def _get_nc():
    if "nc" in _CACHE:
        return _CACHE["nc"]
    nc = bacc.Bacc("TRN2", target_bir_lowering=False, debug=False,
                   num_devices=NCORE)
    d = {}
    d["xn"] = nc.dram_tensor("xn", [P, BPC, CT, L], BF16, kind="ExternalInput")
    d["xtn"] = nc.dram_tensor("xtn", [P, BPC, 2, C], BF16, kind="ExternalInput")
    d["wqt"] = nc.dram_tensor("wqt", [P, CT, C], BF16, kind="ExternalInput")
    d["wkn"] = nc.dram_tensor("wkn", [P, CT, C], BF16, kind="ExternalInput")
    d["wvt"] = nc.dram_tensor("wvt", [P, CT, C], BF16, kind="ExternalInput")
    d["wct"] = nc.dram_tensor("wct", [P, CT, C], BF16, kind="ExternalInput")
    d["qbias"] = nc.dram_tensor("qbias", [1, C], BF16, kind="ExternalInput")
    d["vposc"] = nc.dram_tensor("vposc", [1, C], BF16, kind="ExternalInput")
    d["out"] = nc.dram_tensor("out", [BPC, 2, 512], F32, kind="ExternalOutput")
    with tile.TileContext(nc) as tc, ExitStack() as ctx, \
            nc.allow_low_precision(reason="float32r tiles hold f32 bits"):
        _body(ctx, tc, d)
    nc.compile()
    _CACHE["nc"] = nc
    return nc


def _prep_maps(inputs):
    xf = inputs["x"].reshape(B, C, S2).astype(np.float32)
    pos = inputs["pos_emb"].astype(np.float32)
    xp = xf + pos[None, :, 1:]
    posc = pos[:, 0] - pos[:, 1:].mean(axis=1)          # [C]
    wqkv = inputs["w_qkv"].astype(np.float32)
    wq, wk, wv = wqkv[0:C], wqkv[C:2 * C], wqkv[2 * C:3 * C]
    bq = inputs["b_qkv"][0:C].astype(np.float32)
    bv = inputs["b_qkv"][2 * C:3 * C].astype(np.float32)
    wc = inputs["w_c"].astype(np.float32)
    bc = inputs["b_c"].astype(np.float32)

    bf = ml_dtypes.bfloat16
    wqt = np.ascontiguousarray(
        (wq.T * (SCL / S2)).reshape(CT, P, C).transpose(1, 0, 2)).astype(bf)
    wkn = np.ascontiguousarray(
        wk.reshape(CT, P, C).transpose(1, 0, 2)).astype(bf)
    wvt = np.ascontiguousarray(
        wv.T.reshape(CT, P, C).transpose(1, 0, 2)).astype(bf)
    wct = np.ascontiguousarray(
        wc.T.reshape(CT, P, C).transpose(1, 0, 2)).astype(bf)
    qbias = np.ascontiguousarray((SCL * (wq @ posc + bq))[None, :]).astype(bf)
    vposc = np.ascontiguousarray((wv @ posc)[None, :]).astype(bf)
    poscp = posc.reshape(CT, P).T                        # [P, CT]
    shared = dict(wqt=wqt, wkn=wkn, wvt=wvt, wct=wct,
                  qbias=qbias, vposc=vposc)
    maps = []
    for c in range(NCORE):
        xc = xp[c * BPC:(c + 1) * BPC]                   # [8, 1024, 256]
        m = dict(shared)
        xnc = np.empty((P, BPC, CT, L), dtype=np.float32)
        xnc[:, :, :, 0:S2] = xc.reshape(BPC, CT, P, S2).transpose(2, 0, 1, 3)
        xnc[:, :, :, S2] = poscp[:, None, :]
        m["xn"] = np.ascontiguousarray(xnc).astype(bf)
        m["xtn"] = np.ascontiguousarray(
            xc.reshape(BPC, C, 2, P).transpose(3, 0, 2, 1)).astype(bf)
        maps.append(m)
    _CACHE["hostbias"] = wc @ bv + bc                    # [C]
    return maps


def kernel(**inputs) -> np.ndarray:
    nc = _get_nc()
    maps = _prep_maps(inputs)
    res = run_bass_kernel_spmd(nc, maps, list(range(NCORE)))
    hb = _CACHE["hostbias"]
    outs = []
    for c in range(NCORE):
        arr = res.results[c]["out"].reshape(BPC, C).astype(np.float32)
        outs.append(arr + hb[None, :])
    return np.concatenate(outs, axis=0)


if __name__ == "__main__":
    rng = np.random.default_rng(0)
    ins = {
        "x": rng.standard_normal((B, C, 16, 16), dtype=np.float32),
        "pos_emb": rng.standard_normal((C, L), dtype=np.float32) / 32,
        "w_qkv": rng.standard_normal((3 * C, C), dtype=np.float32) / 32,
        "b_qkv": rng.standard_normal((3 * C,), dtype=np.float32) * 0.1,
        "w_c": rng.standard_normal((C, C), dtype=np.float32) / 32,
        "b_c": rng.standard_normal((C,), dtype=np.float32) * 0.1,
    }
    o = kernel(**ins)
    print("out", o.shape, o.dtype, float(np.abs(o).mean()))


# revision 25
# speedup vs baseline: 1.2810x; 1.0535x over previous
"""AttentionPool2d Trainium2 kernel, 8-core batch-data-parallel (v2).

Math (reference returns only query position 0):
  x' = x + pos_sp  (host-folded), posc = pos0 - mean(pos_sp)
  sums = sum_s x'_s ; xf_m = sums/256 + posc
  q0 = (1/8)(W_q xf_m + b_q)            (the only query needed; 1/8 = attn scale^2)
  u_h = W_k_h^T q0_h  (folds W_k into the query; k never materialized)
  lg_sp = u^T x' ; lg_m = rowsum(lg_sp)/256 + u^T posc
  w = softmax([lg_sp | lg_m]) ; w' = w_sp + w_m/256
  y = x' @ w'^T ; a0 = blockdiag(W_v) y + (W_v posc) w_m
  out = w_c a0 + (w_c b_v + b_c)        (bias added on host)

All-batch (b,h)=128 packed layout after the u stage: one softmax, two PE
transposes, wide-moving matmuls. b_k provably drops out (softmax shift).
"""
import sys
sys.path.insert(0, "/opt/trn_rl_repo")
import numpy as np
import ml_dtypes
from contextlib import ExitStack

from concourse import bacc, tile, mybir
import concourse.bass as bass
from concourse import masks
from concourse.bass_utils import run_bass_kernel_spmd

P = 128
B, C, S2, L = 64, 1024, 256, 257
NH = 16
NCORE, BPC, CT = 8, 8, 8          # cores, batches/core, c-tiles (and q-tiles)
F32R = mybir.dt.float32r
F32 = mybir.dt.float32
BF16 = mybir.dt.bfloat16
AF = mybir.ActivationFunctionType
AX = mybir.AxisListType
OP = mybir.AluOpType
SCL = 1.0 / 8.0                    # (1/ch^0.25)^2 folded into q0


def _body(ctx: ExitStack, tc, d):
    nc = tc.nc
    const = ctx.enter_context(tc.tile_pool(name="const", bufs=1))
    xpool = ctx.enter_context(tc.tile_pool(name="xpool", bufs=1))
    wpool = ctx.enter_context(tc.tile_pool(name="wpool", bufs=1))
    work = ctx.enter_context(tc.tile_pool(name="work", bufs=1))
    acc = ctx.enter_context(tc.tile_pool(name="acc", bufs=1))
    psB = ctx.enter_context(tc.tile_pool(name="psB", bufs=3, space="PSUM"))
    psS = ctx.enter_context(tc.tile_pool(name="psS", bufs=2, space="PSUM"))
    psT = ctx.enter_context(tc.tile_pool(name="psT", bufs=3, space="PSUM"))

    # ---- tiles ----
    xn = xpool.tile([P, BPC, CT, L], BF16)          # x' natural + posc col 256
    xtn = xpool.tile([P, BPC, 2, C], BF16)          # x'^T (s-part)
    wqt = wpool.tile([P, CT, C], BF16)              # (1/2048) W_q^T (c-part, q)
    wkn = wpool.tile([P, CT, C], BF16)              # W_k natural (krow-part, c)
    wvt = wpool.tile([P, CT, C], BF16)              # W_v^T (c-part, vch)
    wct = wpool.tile([P, CT, C], BF16)              # w_c^T (vch-part, o)
    qbias = wpool.tile([1, C], BF16)                # (1/8)(W_q posc + b_q)
    vposc = wpool.tile([1, C], BF16)                # W_v posc

    # ---- DMAs in FIFO priority order (xn first: reduce chain gates q0).
    # Bulk loads issue from the otherwise-idle gpsimd queue; smalls on sync.
    for b in range(BPC):
        nc.sync.dma_start(xn[:, b:b + 1], d["xn"].ap()[:, b:b + 1])
    nc.sync.dma_start(wqt[:], d["wqt"].ap())
    nc.sync.dma_start(qbias[:], d["qbias"].ap())
    nc.sync.dma_start(wkn[:], d["wkn"].ap())
    nc.sync.dma_start(vposc[:], d["vposc"].ap())
    for h in range(2):
        nc.sync.dma_start(xtn[:, 4 * h:4 * h + 4], d["xtn"].ap()[:, 4 * h:4 * h + 4])
    nc.sync.dma_start(wvt[:], d["wvt"].ap())
    nc.sync.dma_start(wct[:], d["wct"].ap())

    identf = const.tile([P, P], F32)
    masks.make_identity(nc, identf[:])
    ident = const.tile([P, P], BF16)
    nc.vector.tensor_copy(ident[:], identf[:])
    ones8 = const.tile([1, BPC], BF16)
    nc.vector.memset(ones8[:], 1.0)

    # ---- stage A: sums over s, xf0 (bf16 pair-add, chunked per batch) ----
    sums = acc.tile([P, BPC, CT], F32R)             # (b, j)
    for b in range(BPC):
        xh = work.tile([P, CT, P], BF16, tag="xh")
        nc.vector.tensor_tensor(xh[:], xn[:, b, :, 0:P],
                                xn[:, b, :, P:S2], op=OP.add)
        nc.vector.reduce_sum(sums[:, b, :], xh[:], axis=AX.X)
    xf0 = acc.tile([P, BPC, CT], BF16)
    nc.vector.tensor_copy(xf0[:], sums[:])

    # ---- q0 (+bias via ones outer-product) ----
    q0p = psS.tile([P, CT, BPC], F32, tag="ps")     # (i, b)
    for i in range(CT):
        for j in range(CT):
            nc.tensor.matmul(q0p[:, i, :], wqt[:, j, i * P:(i + 1) * P],
                             xf0[:, :, j], start=(j == 0), stop=False)
        nc.tensor.matmul(q0p[:, i, :], qbias[0:1, i * P:(i + 1) * P],
                         ones8[:], start=False, stop=True)

    # block-diagonal q0 for the per-head W_k^T fold: col = t*16 + b*2 + h'
    q0blk = acc.tile([P, CT, BPC, 2], BF16)
    nc.vector.memset(q0blk[:], 0.0)
    nc.scalar.activation(q0blk[0:64, :, :, 0], q0p[0:64, :, :], AF.Copy)
    nc.scalar.activation(q0blk[64:P, :, :, 1], q0p[64:P, :, :], AF.Copy)

    # ---- u = blockdiag(W_k)^T q0 ; permuted to (b-major, h) columns ----
    usb = acc.tile([P, CT, BPC, CT, 2], BF16)       # [c-part, j, b, t, h']
    for j in range(CT):
        up = psS.tile([P, CT, BPC, 2], F32, tag="ps")   # (t, b, h')
        for t in range(CT):
            nc.tensor.matmul(up[:, t, :, :], wkn[:, t, j * P:(j + 1) * P],
                             q0blk[:, t, :, :], start=True, stop=True)
        nc.vector.tensor_copy(usb[:, j], up[:].transpose([0, 2, 1, 3]))

    # ---- logits: per-b 16-col stationary into 32-spaced psum blocks ----
    # group g = b//4 holds 4 batches at partition bases 32*(b%4); row =
    # 32*(b%4) + h within a group. Col 256 = u^T posc (posc is x col 256).
    # j-outer, (g,k)-inner: the four 32-col-group positions can overlap on
    # the PE array (column tiling), and each j starts as soon as usb_j lands.
    lgps = [psB.tile([P, 512], F32, tag="pb", name=f"lgp{g}")
            for g in range(2)]
    for g in range(2):
        for j in range(CT):
            for k in range(4):
                b = g * 4 + k
                nc.tensor.matmul(lgps[g][32 * k:32 * k + 16, 0:L],
                                 usb[:, j, b], xn[:, b, j, :],
                                 start=(j == 0), stop=(j == CT - 1),
                                 tile_position=(0, 32 * k))
    lgall = [work.tile([P, L + 3], F32, tag=f"lgall{g}", name=f"lgall{g}")
             for g in range(2)]
    nc.vector.tensor_copy(lgall[0][:, 0:L], lgps[0][:, 0:L])
    nc.scalar.activation(lgall[1][:, 0:L], lgps[1][:, 0:L], AF.Copy)

    # mean-token logit col = rowsum/256 + u^T posc  (then batched softmax)
    wps = []
    wmcols = []
    for g in range(2):
        lg = lgall[g]
        rowsum = work.tile([P, 1], F32, tag=f"rs{g}")
        nc.vector.reduce_sum(rowsum[:], lg[:, 0:S2], axis=AX.X)
        nc.vector.tensor_scalar(lg[:, S2:S2 + 1], rowsum[:], 1.0 / S2,
                                lg[:, S2:S2 + 1], op0=OP.mult, op1=OP.add)
        negmax = work.tile([P, 1], F32, tag=f"nm{g}")
        nc.vector.reduce_max(negmax[:], lg[:, 0:L], axis=AX.X, negate=True)
        ex = work.tile([P, L], F32R, tag=f"ex{g}")
        sumexp = work.tile([P, 1], F32R, tag=f"se{g}")
        nc.scalar.activation(ex[:], lg[:, 0:L], AF.Exp, bias=negmax[:],
                             accum_out=sumexp[:])
        recip = work.tile([P, 1], F32, tag=f"rc{g}")
        nc.vector.reciprocal(recip[:], sumexp[:])
        emdiv = work.tile([P, 1], F32, tag=f"ed{g}")
        nc.vector.tensor_scalar_mul(emdiv[:], ex[:, S2:L], 1.0 / S2)
        wp = work.tile([P, S2], BF16, tag=f"wp{g}")  # w' = (e_sp + e_m/256) r
        nc.vector.tensor_scalar(wp[:], ex[:, 0:S2], emdiv[:], recip[:],
                                op0=OP.add, op1=OP.mult)
        wmcol = work.tile([P, 1], BF16, tag=f"wm{g}")  # w_m = e_m r
        nc.vector.tensor_scalar(wmcol[:], ex[:, S2:L], recip[:], None,
                                op0=OP.mult)
        wps.append(wp)
        wmcols.append(wmcol)

    # ---- transposes: w'^T chunks (s-part, 32-spaced (b,h)) + w_m row ----
    wtas = []
    wmrow = work.tile([1, BPC, 16], BF16)
    for g in range(2):
        wta = acc.tile([P, 2, P], BF16, tag=f"wta{g}")
        for t2 in range(2):
            tp2 = psT.tile([P, P], BF16, tag="pt")
            nc.tensor.transpose(tp2[:], wps[g][:, t2 * P:(t2 + 1) * P],
                                ident[:])
            nc.vector.tensor_copy(wta[:, t2], tp2[:])
        wtas.append(wta)
        wmp = psT.tile([1, 4, 32], BF16, tag="pt")
        nc.tensor.transpose(wmp[0:1], wmcols[g][:], ident[:])
        nc.vector.tensor_copy(wmrow[0:1, g * 4:(g + 1) * 4, :],
                              wmp[0:1, :, 0:16])

    # ---- y^T = w'^T-stationary @ x'^T  -> 32-spaced [(b,h), c], then
    # transpose back per group so a0 can chase group 1's transposes ----
    yTs = [acc.tile([P, C], BF16, tag=f"yT{g}", name=f"yT{g}")
           for g in range(2)]
    yall = acc.tile([P, CT, BPC, 16], BF16)
    cnt = 0
    for g in range(2):
        for k in range(4):
            b = g * 4 + k
            for half in range(2):
                yp = psB.tile([P, 512], F32, tag="pb")
                for t2 in range(2):
                    nc.tensor.matmul(yp[:], wtas[g][:, t2],
                                     xtn[:, b, t2, half * 512:(half + 1) * 512],
                                     start=(t2 == 0), stop=(t2 == 1))
                cnt += 1
                dst = yTs[g][32 * k:32 * k + 16, half * 512:(half + 1) * 512]
                src = yp[32 * k:32 * k + 16, 0:512]
                if cnt % 2 == 0:
                    nc.scalar.activation(dst, src, AF.Copy)
                else:
                    nc.vector.tensor_copy(dst, src)
        for j in range(CT):
            tp = psT.tile([P, 4, 32], BF16, tag="pt")
            nc.tensor.transpose(tp[:], yTs[g][:, j * P:(j + 1) * P], ident[:])
            if j % 2 == 0:
                nc.vector.tensor_copy(yall[:, j, g * 4:(g + 1) * 4, :],
                                      tp[:, :, 0:16])
            else:
                nc.scalar.activation(yall[:, j, g * 4:(g + 1) * 4, :],
                                     tp[:, :, 0:16], AF.Copy)

    # ---- a0 = blockdiag(W_v) y + vposc * w_m ----
    a0ps = psS.tile([P, CT, BPC, 2], F32, tag="ps")
    for r in range(CT):
        for j in range(CT):
            nc.tensor.matmul(a0ps[:, r, :, :], wvt[:, j, r * P:(r + 1) * P],
                             yall[:, j, :, 2 * r:2 * r + 2],
                             start=(j == 0), stop=False)
        nc.tensor.matmul(a0ps[:, r, :, :], vposc[0:1, r * P:(r + 1) * P],
                         wmrow[0:1, :, 2 * r:2 * r + 2], start=False, stop=True)
    a0blk = acc.tile([P, CT, BPC], BF16)            # (vch-part, (r, b))
    nc.scalar.activation(a0blk[0:64, :, :], a0ps[0:64, :, :, 0], AF.Copy)
    nc.scalar.activation(a0blk[64:P, :, :], a0ps[64:P, :, :, 1], AF.Copy)

    # ---- out^T = a0-stationary @ w_c^T  -> [b, o] ----
    osb = acc.tile([BPC, 2, 512], F32)
    for half in range(2):
        op = psB.tile([P, 512], F32, tag="pb")
        for r in range(CT):
            nc.tensor.matmul(op[0:BPC, :], a0blk[:, r, :],
                             wct[:, r, half * 512:(half + 1) * 512],
                             start=(r == 0), stop=(r == CT - 1))
        nc.vector.tensor_copy(osb[:, half, :], op[0:BPC, :])
    nc.sync.dma_start(d["out"].ap(), osb[:])


_CACHE = {}


def _get_nc():
    if "nc" in _CACHE:
        return _CACHE["nc"]
    nc = bacc.Bacc("TRN2", target_bir_lowering=False, debug=False,
                   num_devices=NCORE)
    d = {}
    d["xn"] = nc.dram_tensor("xn", [P, BPC, CT, L], BF16, kind="ExternalInput")
    d["xtn"] = nc.dram_tensor("xtn", [P, BPC, 2, C], BF16, kind="ExternalInput")
    d["wqt"] = nc.dram_tensor("wqt", [P, CT, C], BF16, kind="ExternalInput")
    d["wkn"] = nc.dram_tensor("wkn", [P, CT, C], BF16, kind="ExternalInput")
    d["wvt"] = nc.dram_tensor("wvt", [P, CT, C], BF16, kind="ExternalInput")
    d["wct"] = nc.dram_tensor("wct", [P, CT, C], BF16, kind="ExternalInput")
    d["qbias"] = nc.dram_tensor("qbias", [1, C], BF16, kind="ExternalInput")
    d["vposc"] = nc.dram_tensor("vposc", [1, C], BF16, kind="ExternalInput")
    d["out"] = nc.dram_tensor("out", [BPC, 2, 512], F32, kind="ExternalOutput")
    with tile.TileContext(nc) as tc, ExitStack() as ctx, \
            nc.allow_low_precision(reason="float32r tiles hold f32 bits"):
        _body(ctx, tc, d)
    nc.compile()
    _CACHE["nc"] = nc
    return nc


def _prep_maps(inputs):
    xf = inputs["x"].reshape(B, C, S2).astype(np.float32)
    pos = inputs["pos_emb"].astype(np.float32)
    xp = xf + pos[None, :, 1:]
    posc = pos[:, 0] - pos[:, 1:].mean(axis=1)          # [C]
    wqkv = inputs["w_qkv"].astype(np.float32)
    wq, wk, wv = wqkv[0:C], wqkv[C:2 * C], wqkv[2 * C:3 * C]
    bq = inputs["b_qkv"][0:C].astype(np.float32)
    bv = inputs["b_qkv"][2 * C:3 * C].astype(np.float32)
    wc = inputs["w_c"].astype(np.float32)
    bc = inputs["b_c"].astype(np.float32)

    bf = ml_dtypes.bfloat16
    wqt = np.ascontiguousarray(
        (wq.T * (SCL / S2)).reshape(CT, P, C).transpose(1, 0, 2)).astype(bf)
    wkn = np.ascontiguousarray(
        wk.reshape(CT, P, C).transpose(1, 0, 2)).astype(bf)
    wvt = np.ascontiguousarray(
        wv.T.reshape(CT, P, C).transpose(1, 0, 2)).astype(bf)
    wct = np.ascontiguousarray(
        wc.T.reshape(CT, P, C).transpose(1, 0, 2)).astype(bf)
    qbias = np.ascontiguousarray((SCL * (wq @ posc + bq))[None, :]).astype(bf)
    vposc = np.ascontiguousarray((wv @ posc)[None, :]).astype(bf)
    poscp = posc.reshape(CT, P).T                        # [P, CT]
    shared = dict(wqt=wqt, wkn=wkn, wvt=wvt, wct=wct,
                  qbias=qbias, vposc=vposc)
    maps = []
    for c in range(NCORE):
        xc = xp[c * BPC:(c + 1) * BPC]                   # [8, 1024, 256]
        m = dict(shared)
        xnc = np.empty((P, BPC, CT, L), dtype=np.float32)
        xnc[:, :, :, 0:S2] = xc.reshape(BPC, CT, P, S2).transpose(2, 0, 1, 3)
        xnc[:, :, :, S2] = poscp[:, None, :]
        m["xn"] = np.ascontiguousarray(xnc).astype(bf)
        m["xtn"] = np.ascontiguousarray(
            xc.reshape(BPC, C, 2, P).transpose(3, 0, 2, 1)).astype(bf)
        maps.append(m)
    _CACHE["hostbias"] = wc @ bv + bc                    # [C]
    return maps


def kernel(**inputs) -> np.ndarray:
    nc = _get_nc()
    maps = _prep_maps(inputs)
    res = run_bass_kernel_spmd(nc, maps, list(range(NCORE)))
    hb = _CACHE["hostbias"]
    outs = []
    for c in range(NCORE):
        arr = res.results[c]["out"].reshape(BPC, C).astype(np.float32)
        outs.append(arr + hb[None, :])
    return np.concatenate(outs, axis=0)


if __name__ == "__main__":
    rng = np.random.default_rng(0)
    ins = {
        "x": rng.standard_normal((B, C, 16, 16), dtype=np.float32),
        "pos_emb": rng.standard_normal((C, L), dtype=np.float32) / 32,
        "w_qkv": rng.standard_normal((3 * C, C), dtype=np.float32) / 32,
        "b_qkv": rng.standard_normal((3 * C,), dtype=np.float32) * 0.1,
        "w_c": rng.standard_normal((C, C), dtype=np.float32) / 32,
        "b_c": rng.standard_normal((C,), dtype=np.float32) * 0.1,
    }
    o = kernel(**ins)
    print("out", o.shape, o.dtype, float(np.abs(o).mean()))
